# revision 24
# baseline (speedup 1.0000x reference)
"""ComplexAttentionV3 Trainium2 kernel (v13).

Sharding: 8 cores = data-parallel over batch (2) x tensor-parallel over
heads (16 -> 4 per core). Each core computes q/k/v for its 4 heads
(column-sharded projections), local attention, and a row-sharded
o-projection producing a partial [T, D] output; the host sums the 4
partials per batch.

v13 notes vs v9 (402us -> 382us):
- q/k/v projections use the 3-multiplication Karatsuba form of the
  complex matmul: m1 = xr@wr, m2 = xi@wi, m3 = (xr+xi)@(wr+wi);
  re = m1-m2, im = (m3-m1)-m2. Projection-phase PE column-streams drop
  25%. Combines run on the DVE (one PSUM operand per op -- m2 staged
  to SBUF first; NOT on the scalar engine, whose instruction stream
  sits behind the flow-controlled DMA issuances). xs = xr+xi on DVE.
- proj loop is quarter-interleaved (qk groups + v chunks per 512-col
  quarter); x lives in rolling SBUF buffers 3 quarters deep. The
  wrap-around quarter's DMA is emitted only after the quarter whose
  slot it reuses, or Tile wires the early reads to the late write.
- quarter-0 startup: the first two q groups' m1/m2/m3 runs interleave
  so the PE has resident-xr work while xi lands.
- attention runs iw-outer/h-inner; after window iw=0 the first-half
  o-projection is decomposed into single-bank [128,512] PSUM pieces
  interleaved ~6 per window into iw=1's jc stream (PE slack under the
  exp-bound window is ~260ns/jc -- denser insertion stalls the exp
  stream). The rest runs as a tail on a 4-deep PSUM ring carved out
  of the released scores/av banks.
- softmax denominator: bf16 esum accumulated on DVE; 2 ones-matmuls
  into separate one-bank tiles (a partition-64 matmul output lands
  wrong on HW); finisher (dn->rec->bc->mul) emitted a few jc into the
  next window. gpsimd broadcast warmed up at kernel start; gpsimd
  cannot read PSUM and its Q7 adds are ~2x DVE, so it only runs the
  partition broadcasts.
- outputs bf16: real + in-window imag on the SP queue, tail imag on
  the ACT queue.
"""

import numpy as np
import ml_dtypes

import concourse.bacc as bacc
import concourse.tile as tile
from concourse import mybir
from concourse.bass import ts
from concourse.bass_utils import run_bass_kernel_spmd

B, T, D, H = 2, 2048, 1024, 16
HD = 64
NCORE = 8
TP = 4               # head-parallel degree (per batch)
HC = H // TP         # heads per core = 4
C = HC * HD          # local channels = 256
DC = D // 128        # contraction chunks = 8
TQ = T // 128        # 128-row t-chunks = 16
TW = T // 1024       # 1024-col t-chunks = 2
QT = T // 512        # 512-col quarters = 4
XR_SLOTS = 3         # rolling depth of the x quarter buffers
XS_SLOTS = 2

F32 = mybir.dt.float32
BF16 = mybir.dt.bfloat16
EXP = mybir.ActivationFunctionType.Exp

LAST_RESULTS = None
_COMPILED = None


def _build():
    nc = bacc.Bacc("TRN2", target_bir_lowering=False, debug=False,
                   num_devices=NCORE)

    def din(name, shape, dt=BF16):
        return nc.dram_tensor(name, shape, dt, kind="ExternalInput").ap()

    xr_d = din("xrT", [128, DC, T])
    xi_d = din("xiT", [128, DC, T])
    wq = {k: din(f"wq_{k}", [128, DC, C]) for k in ("r", "i", "s")}
    wk = {k: din(f"wk_{k}", [128, DC, C]) for k in ("r", "i", "s")}
    wv = {k: din(f"wv_{k}", [128, DC, C]) for k in ("r", "i", "s")}
    ow = {k: din(f"ow_{k}", [128, 2, D]) for k in ("r", "i", "n")}
    cos_d = din("cos2", [128, T], BF16)
    sin_d = din("sin2", [128, T], BF16)
    outr_d = nc.dram_tensor("out_r", [T, D], BF16, kind="ExternalOutput").ap()
    outi_d = nc.dram_tensor("out_i", [T, D], BF16, kind="ExternalOutput").ap()

    with tile.TileContext(nc) as tc:
        with tc.tile_pool(name="persist", bufs=1) as persist:
            qkcat = persist.tile([128, 2 * HC, T], BF16, name="qkcat")
            vcat = persist.tile([128, TQ, HC, 128], BF16, name="vcat")
            urt = persist.tile([128, 2, T], BF16, name="urt")
            uit = persist.tile([128, 2, T], BF16, name="uit")
            ones = persist.tile([128, 1], BF16, name="ones")
            nc.vector.memset(ones[:], 1.0)
            # dummy broadcast: preloads the gpsimd program while the
            # engine is idle (first dispatch otherwise costs ~7.5us in
            # the middle of the attention phase)
            bwarm_in = persist.tile([1, 8], F32, name="bwarm_in")
            bwarm = persist.tile([128, 8], F32, name="bwarm")
            nc.vector.memset(bwarm_in[:], 1.0)
            nc.gpsimd.partition_broadcast(bwarm[:], bwarm_in[:])

            # -------- input SBUF --------
            xw = tc.alloc_tile_pool(name="xw", bufs=1)
            wqs = {k: xw.tile([128, DC, C], BF16, name=f"wq{k}")
                   for k in ("r", "i", "s")}
            wks = {k: xw.tile([128, DC, C], BF16, name=f"wk{k}")
                   for k in ("r", "i", "s")}
            wvs = {k: xw.tile([128, DC, C], BF16, name=f"wv{k}")
                   for k in ("r", "i", "s")}
            cos = xw.tile([128, T], BF16, name="cos")
            sin = xw.tile([128, T], BF16, name="sin")
            xr = xw.tile([128, XR_SLOTS, DC, 512], BF16, name="xr")
            xi = xw.tile([128, XR_SLOTS, DC, 512], BF16, name="xi")
            xs = xw.tile([128, XS_SLOTS, DC, 512], BF16, name="xs")

            # -------- input DMA: ordered by first consumer --------
            # ACT queue: q weights (first matmul group), then per
            # quarter the xi pieces, with k/v weights slotted after
            # the quarter that gives them enough lead time.
            for dc in range(DC):
                nc.scalar.dma_start(wqs["r"][:, dc:dc + 1],
                                    wq["r"][:, dc:dc + 1])
            nc.scalar.dma_start(wqs["i"][:], wq["i"][:])

            def xi_quarter(q):
                for dc in range(DC):
                    nc.scalar.dma_start(xi[:, q % XR_SLOTS, dc, :],
                                        xi_d[:, dc, ts(q, 512)])

            # NOTE: only the first XR_SLOTS quarters are DMA'd up front.
            # The wrap-around quarter (q=3 reuses slot 0) must be emitted
            # AFTER quarter 0's compute, or Tile wires quarter-0's reads
            # to the later write (last-writer RAW) and the kernel computes
            # on the wrong data.
            xi_quarter(0)
            nc.scalar.dma_start(wqs["s"][:], wq["s"][:])
            for k in ("r", "i", "s"):
                nc.scalar.dma_start(wks[k][:], wk[k][:])
            xi_quarter(1)
            for k in ("r", "i", "s"):
                nc.scalar.dma_start(wvs[k][:], wv[k][:])
            xi_quarter(2)

            # SP queue: xr quarters with their rope-table chunks just
            # behind, o-projection weights at the tail.
            def xr_quarter(q):
                for dc in range(DC):
                    nc.sync.dma_start(xr[:, q % XR_SLOTS, dc, :],
                                      xr_d[:, dc, ts(q, 512)])

            for q in range(XR_SLOTS):
                xr_quarter(q)
                qs = ts(q, 512)
                nc.sync.dma_start(cos[:, qs], cos_d[:, qs])
                nc.sync.dma_start(sin[:, qs], sin_d[:, qs])
            qs = ts(3, 512)
            nc.sync.dma_start(cos[:, qs], cos_d[:, qs])
            nc.sync.dma_start(sin[:, qs], sin_d[:, qs])
            ows = {k: persist.tile([128, 2, D], BF16, name=f"ow{k}")
                   for k in ("r", "i", "n")}
            for k in ("r", "i", "n"):
                nc.sync.dma_start(ows[k][:], ow[k][:])

            # ---------------- projection phase ----------------
            # Karatsuba: m1 = xr@wr, m2 = xi@wi, m3 = xs@ws;
            # re = m1-m2, im = (m3-m1)-m2.
            with tc.tile_pool(name="rt", bufs=1) as rt, \
                 tc.tile_pool(name="pj", bufs=2, space="PSUM") as pj:

                def qk_mrun(pm, wsrc, cc, q, m):
                    slot = q % XR_SLOTS
                    xbuf, wkey = ((xr, "r"), (xi, "i"))[m] if m < 2 \
                        else (None, "s")
                    for dc in range(DC):
                        rhs = (xs[:, q % XS_SLOTS, dc, :] if m == 2
                               else xbuf[:, slot, dc, :])
                        nc.tensor.matmul(pm[:, m, :],
                                         lhsT=wsrc[wkey][:, dc, ts(cc, 128)],
                                         rhs=rhs,
                                         start=(dc == 0),
                                         stop=(dc == DC - 1))

                def qk_group(wsrc, hbase, cc, q, pm=None):
                    hsl = ts(q, 512)
                    if pm is None:
                        pm = pj.tile([128, 3, 512], F32, name="pm")
                        for m in range(3):
                            qk_mrun(pm, wsrc, cc, q, m)
                    # DVE reads at most one PSUM operand per op: stage m2
                    # to SBUF first. NOT on the scalar engine: its
                    # instruction stream sits behind the flow-controlled
                    # DMA issuances and runs ~45us late.
                    qrh = rt.tile([128, 512], BF16, name="qrh")
                    qih = rt.tile([128, 512], BF16, name="qih")
                    uu = rt.tile([128, 512], F32, name="uu")
                    m2c = rt.tile([128, 512], F32, name="m2c")
                    nc.vector.tensor_copy(m2c[:], pm[:, 1, :])
                    nc.vector.tensor_sub(qrh[:], pm[:, 0, :], m2c[:])
                    nc.vector.tensor_sub(uu[:], pm[:, 2, :], m2c[:])
                    nc.vector.tensor_sub(qih[:], uu[:], pm[:, 0, :])
                    # rope
                    h0, h1 = hbase + 2 * cc, hbase + 2 * cc + 1
                    t1 = rt.tile([128, 512], BF16, name="t1")
                    t2 = rt.tile([128, 512], BF16, name="t2")
                    t3 = rt.tile([128, 512], BF16, name="t3")
                    t4 = rt.tile([128, 512], BF16, name="t4")
                    nc.vector.tensor_mul(t1[:], qrh[:], cos[:, hsl])
                    nc.vector.tensor_mul(t2[:], qih[:], sin[:, hsl])
                    nc.vector.tensor_mul(t3[:], qrh[:], sin[:, hsl])
                    nc.vector.tensor_mul(t4[:], qih[:], cos[:, hsl])
                    nc.vector.tensor_sub(qkcat[0:64, h0, hsl],
                                         t1[0:64, :], t2[0:64, :])
                    nc.vector.tensor_sub(qkcat[0:64, h1, hsl],
                                         t1[64:128, :], t2[64:128, :])
                    nc.vector.tensor_add(qkcat[64:128, h0, hsl],
                                         t3[0:64, :], t4[0:64, :])
                    nc.vector.tensor_add(qkcat[64:128, h1, hsl],
                                         t3[64:128, :], t4[64:128, :])

                def v_group(tq):
                    q, k = tq // 4, tq % 4
                    slot = q % XR_SLOTS
                    ksl = ts(k, 128)
                    pm = pj.tile([128, 3, 512], F32, name="pm")
                    vsl = ts(0, 256)
                    for m, xbuf, wkey in ((0, xr, "r"), (1, xi, "i")):
                        for dc in range(DC):
                            nc.tensor.matmul(pm[:, m, vsl],
                                             lhsT=xbuf[:, slot, dc, ksl],
                                             rhs=wvs[wkey][:, dc, :],
                                             start=(dc == 0),
                                             stop=(dc == DC - 1))
                    for dc in range(DC):
                        nc.tensor.matmul(pm[:, 2, vsl],
                                         lhsT=xs[:, q % XS_SLOTS, dc, ksl],
                                         rhs=wvs["s"][:, dc, :],
                                         start=(dc == 0),
                                         stop=(dc == DC - 1))
                    vt = rt.tile([128, C], F32, name="vt")
                    vm2c = rt.tile([128, C], F32, name="vm2c")
                    nc.vector.tensor_copy(vm2c[:], pm[:, 1, vsl])
                    nc.vector.tensor_sub(
                        vcat[:, tq, :, 0:64],
                        pm[:, 0, vsl].rearrange("p (h d) -> p h d", h=HC),
                        vm2c[:].rearrange("p (h d) -> p h d", h=HC))
                    nc.vector.tensor_sub(vt[:], pm[:, 2, vsl], vm2c[:])
                    nc.vector.tensor_sub(
                        vcat[:, tq, :, 64:128],
                        vt[:].rearrange("p (h d) -> p h d", h=HC),
                        pm[:, 0, vsl].rearrange("p (h d) -> p h d", h=HC))

                def xs_adds(q):
                    # all on DVE: gpsimd's slower Q7 adds end up gating
                    # the m3 runs even with early emission
                    for dc in range(DC):
                        nc.vector.tensor_add(xs[:, q % XS_SLOTS, dc, :],
                                             xr[:, q % XR_SLOTS, dc, :],
                                             xi[:, q % XR_SLOTS, dc, :])

                xs_adds(0)
                for q in range(QT):
                    if q == 0:
                        # startup: the first two groups' m1/m2/m3 runs
                        # interleave so the PE has resident-xr work
                        # while xi lands and the xs adds catch up
                        pms = [pj.tile([128, 3, 512], F32, name="pm")
                               for _ in range(2)]
                        for m in range(3):
                            for g in range(2):
                                qk_mrun(pms[g], wqs, g, 0, m)
                        for g in range(2):
                            qk_group(wqs, 0, g, 0, pm=pms[g])
                        xs_adds(1)
                        for cc in range(2):
                            qk_group(wks, HC, cc, 0)
                    else:
                        done = 0
                        for wsrc, hbase in ((wqs, 0), (wks, HC)):
                            for cc in range(2):
                                qk_group(wsrc, hbase, cc, q)
                                done += 1
                                if done == 2 and q + 1 < QT:
                                    # next quarter's xs adds emitted
                                    # mid-quarter so they clear the DVE
                                    # queue before its m3 groups run
                                    xs_adds(q + 1)
                    for k in range(4):
                        v_group(4 * q + k)
                    # rolling-buffer refill: quarter q+XR_SLOTS reuses
                    # this quarter's slot, so its DMA is emitted only now
                    if q + XR_SLOTS < QT:
                        xr_quarter(q + XR_SLOTS)
                        xi_quarter(q + XR_SLOTS)

            # x and q/k/v weights are consumed; free their SBUF before
            # opening the attention pools.
            xw.release()

            # PSUM pools stack-ordered so mm/avp (released before the
            # o-proj tail) sit on top
            op = tc.alloc_tile_pool(name="op", bufs=2, space="PSUM")
            avp = tc.alloc_tile_pool(name="avp", bufs=1, space="PSUM")
            mm = tc.alloc_tile_pool(name="mm", bufs=2, space="PSUM")
            att = tc.alloc_tile_pool(name="att", bufs=6)
            asm = tc.alloc_tile_pool(name="asm", bufs=2)
            ost = tc.alloc_tile_pool(name="ost", bufs=4)

            # ---------------- attention + o-projection ----------------
            # iw-outer / h-inner: after window iw=0 finishes all 4 heads,
            # the o-projection for query rows 0:1024 is unblocked; its
            # 64 single-bank PSUM pieces are interleaved into iw=1's jc
            # stream to fill the PE slack under the exp-bound window.
            # The per-window softmax finisher (dn -> rec -> bc -> muls)
            # is emitted a few jc iterations INTO the next window so the
            # chain pipelines under exp.
            pend = None  # finisher state of the previous window

            def fin_dn(p):
                # one [128,512] op-ring slot per 512-half, each written
                # at partition 0 (a partition-64 matmul output lands
                # wrong on HW). Same name/shape as the o-proj pieces so
                # the pool keeps a single 2-buffer ring.
                dns = []
                for half in range(2):
                    dn = op.tile([128, 512], F32, name="po")
                    nc.tensor.matmul(dn[0:1, :], lhsT=ones[:],
                                     rhs=p["esum"][:, ts(half, 512)],
                                     start=True, stop=True)
                    dns.append(dn)
                p["dn"] = dns

            def fin_rec(p):
                rec = asm.tile([1, 1024], F32, name="rec")
                for half in range(2):
                    nc.vector.reciprocal_approx_fast(
                        rec[:, ts(half, 512)], p["dn"][half][0:1, :])
                p["rec"] = rec

            def fin_bc(p):
                bc = asm.tile([128, 1024], F32, name="bc")
                nc.gpsimd.partition_broadcast(bc[:], p["rec"][:])
                p["bc"] = bc

            def fin_mul(p):
                ucc, up0, isl = p["ucc"], p["up0"], p["isl"]
                nc.vector.tensor_mul(urt[up0:up0 + 64, ucc, isl],
                                     p["avr"][0:64, :], p["bc"][0:64, :])
                nc.vector.tensor_mul(uit[up0:up0 + 64, ucc, isl],
                                     p["avr"][64:128, :], p["bc"][64:128, :])

            opool = [None]

            def o_piece(tq, oc, im):
                def emit(tail=False):
                    tslq, osl = ts(tq, 128), ts(oc, 512)
                    po = opool[0].tile([128, 512], F32, name="po")
                    if im == 0:
                        seq = ((urt, 0, "r"), (urt, 1, "r"),
                               (uit, 0, "n"), (uit, 1, "n"))
                        dst = outr_d
                    else:
                        seq = ((urt, 0, "i"), (urt, 1, "i"),
                               (uit, 0, "r"), (uit, 1, "r"))
                        dst = outi_d
                    for n, (ut, ch, wk2) in enumerate(seq):
                        nc.tensor.matmul(po[:], lhsT=ut[:, ch, tslq],
                                         rhs=ows[wk2][:, ch, osl],
                                         start=(n == 0), stop=(n == 3))
                    stp = ost.tile([128, 512], BF16, name="stp")
                    nc.vector.tensor_copy(stp[:], po[:])
                    # imag tail pieces drain on the (then idle) ACT
                    # queue; everything else on SP
                    if im == 1 and tq >= 8:
                        nc.scalar.dma_start(dst[tslq, osl], stp[:])
                    else:
                        nc.sync.dma_start(dst[tslq, osl], stp[:])
                return emit

            OSLOT = {4, 6, 8, 10, 12, 14}
            opq = []
            opool[0] = op
            for iw in range(TW):
                isl = ts(iw, 1024)
                if iw == 1:
                    opq = [o_piece(tq, oc, im)
                           for tq in range(8) for im in range(2)
                           for oc in range(2)]
                for h in range(HC):
                    ucc, up0 = h // 2, (h % 2) * 64
                    av = avp.tile([128, 1024], F32, name="av")
                    esum = asm.tile([128, 1024], BF16, name="esum")
                    for jc in range(TQ):
                        s = mm.tile([128, 1024], F32, name="mmt")
                        for half in range(2):
                            nc.tensor.matmul(
                                s[:, ts(half, 512)],
                                lhsT=qkcat[:, HC + h, ts(jc, 128)],
                                rhs=qkcat[:, h, ts(2 * iw + half, 512)],
                                start=True, stop=True)
                        es = att.tile([128, 1024], BF16, name="es")
                        nc.scalar.activation(es[:], s[:], EXP, scale=0.125)
                        for half in range(2):
                            psl = ts(half, 512)
                            nc.tensor.matmul(av[:, psl],
                                             lhsT=vcat[:, jc, h, :],
                                             rhs=es[:, psl],
                                             start=(jc == 0),
                                             stop=(jc == TQ - 1))
                        if jc == 0:
                            nc.vector.tensor_copy(esum[:], es[:])
                        else:
                            nc.vector.tensor_add(esum[:], esum[:], es[:])
                        if pend is not None:
                            if jc == 1:
                                fin_dn(pend)
                            elif jc == 2:
                                fin_rec(pend)
                            elif jc == 3:
                                fin_bc(pend)
                            elif jc == 5:
                                fin_mul(pend)
                                pend = None
                        if opq and jc in OSLOT and (h > 0 or jc >= 8):
                            opq.pop(0)()
                    avr = asm.tile([128, 1024], BF16, name="avr")
                    nc.vector.tensor_copy(avr[:], av[:])
                    pend = {"esum": esum, "avr": avr,
                            "ucc": ucc, "up0": up0, "isl": isl}
            # flush the final window's finisher, then the second-half
            # o-projection (query rows 1024:2048)
            fin_dn(pend)
            fin_rec(pend)
            fin_bc(pend)
            fin_mul(pend)
            pend = None
            # tail o-projection: scores/av PSUM is dead, so re-use its
            # banks as a deeper 4-slot ring (drain-WAR no longer paces
            # the PE)
            mm.release()
            avp.release()
            otail = tc.alloc_tile_pool(name="otail", bufs=4, space="PSUM")
            opool[0] = otail
            while opq:          # pieces that found no insertion slot
                opq.pop(0)(True)
            for tq in range(8, 16):
                for im in range(2):
                    for oc in range(2):
                        o_piece(tq, oc, im)(True)

            ost.release()
            asm.release()
            att.release()
            otail.release()
            op.release()

    nc.compile()
    return nc


def _to_bf16_kxm(arr, parts=128):
    """[K, M] fp32 -> [128, K//128, M] bf16 with K split as (chunk, part)."""
    k, m = arr.shape
    out = arr.reshape(k // parts, parts, m).transpose(1, 0, 2)
    return np.ascontiguousarray(out.astype(ml_dtypes.bfloat16))


def _rope_tables():
    inv_freq = 1.0 / (10000.0 ** (np.arange(0, HD, 2, dtype=np.float64) / HD))
    invf64 = np.concatenate([inv_freq, inv_freq])          # [64]
    ang = invf64[:, None] * np.arange(T, dtype=np.float64)[None, :]  # [64, T]
    cos2 = np.tile(np.cos(ang), (2, 1)).astype(ml_dtypes.bfloat16)
    sin2 = np.tile(np.sin(ang), (2, 1)).astype(ml_dtypes.bfloat16)
    return np.ascontiguousarray(cos2), np.ascontiguousarray(sin2)


def kernel(x_real, x_imag, q_wr, q_wi, k_wr, k_wi, v_wr, v_wi, o_wr, o_wi):
    global _COMPILED, LAST_RESULTS
    if _COMPILED is None:
        _COMPILED = _build()
    nc = _COMPILED

    cos2, sin2 = _rope_tables()
    xt = {}
    for b in range(B):
        xt[("r", b)] = _to_bf16_kxm(np.asarray(x_real[b]).T.astype(np.float32))
        xt[("i", b)] = _to_bf16_kxm(np.asarray(x_imag[b]).T.astype(np.float32))

    in_maps = []
    for core in range(NCORE):
        b, g = core // TP, core % TP
        cols = slice(g * C, (g + 1) * C)
        m = {"xrT": xt[("r", b)], "xiT": xt[("i", b)],
             "cos2": cos2, "sin2": sin2}
        for nm, wr_, wi_ in (("wq", q_wr, q_wi), ("wk", k_wr, k_wi),
                             ("wv", v_wr, v_wi)):
            wrc = np.asarray(wr_[:, cols])
            wic = np.asarray(wi_[:, cols])
            m[f"{nm}_r"] = _to_bf16_kxm(wrc)
            m[f"{nm}_i"] = _to_bf16_kxm(wic)
            m[f"{nm}_s"] = _to_bf16_kxm(wrc + wic)
        m["ow_r"] = _to_bf16_kxm(np.asarray(o_wr[cols, :]))
        m["ow_i"] = _to_bf16_kxm(np.asarray(o_wi[cols, :]))
        m["ow_n"] = _to_bf16_kxm(-np.asarray(o_wi[cols, :]))
        in_maps.append(m)

    res = run_bass_kernel_spmd(nc, in_maps, core_ids=list(range(NCORE)))
    LAST_RESULTS = res

    final_r = np.zeros((B, T, D), np.float32)
    final_i = np.zeros((B, T, D), np.float32)
    for core in range(NCORE):
        b = core // TP
        final_r[b] += np.asarray(res.results[core]["out_r"],
                                 dtype=np.float32)
        final_i[b] += np.asarray(res.results[core]["out_i"],
                                 dtype=np.float32)
    return final_r, final_i


# revision 26
# speedup vs baseline: 1.1740x; 1.1740x over previous
"""ComplexAttentionV3 Trainium2 kernel (v13).

Sharding: 8 cores = data-parallel over batch (2) x tensor-parallel over
heads (16 -> 4 per core). Each core computes q/k/v for its 4 heads
(column-sharded projections), local attention, and a row-sharded
o-projection producing a partial [T, D] output; the host sums the 4
partials per batch.

v13 notes vs v9 (402us -> 382us):
- q/k/v projections use the 3-multiplication Karatsuba form of the
  complex matmul: m1 = xr@wr, m2 = xi@wi, m3 = (xr+xi)@(wr+wi);
  re = m1-m2, im = (m3-m1)-m2. Projection-phase PE column-streams drop
  25%. Combines run on the DVE (one PSUM operand per op -- m2 staged
  to SBUF first; NOT on the scalar engine, whose instruction stream
  sits behind the flow-controlled DMA issuances). xs = xr+xi on DVE.
- proj loop is quarter-interleaved (qk groups + v chunks per 512-col
  quarter); x lives in rolling SBUF buffers 3 quarters deep. The
  wrap-around quarter's DMA is emitted only after the quarter whose
  slot it reuses, or Tile wires the early reads to the late write.
- quarter-0 startup: the first two q groups' m1/m2/m3 runs interleave
  so the PE has resident-xr work while xi lands.
- attention runs iw-outer/h-inner; after window iw=0 the first-half
  o-projection is decomposed into single-bank [128,512] PSUM pieces
  interleaved ~6 per window into iw=1's jc stream (PE slack under the
  exp-bound window is ~260ns/jc -- denser insertion stalls the exp
  stream). The rest runs as a tail on a 4-deep PSUM ring carved out
  of the released scores/av banks.
- softmax denominator: bf16 esum accumulated on DVE; 2 ones-matmuls
  into separate one-bank tiles (a partition-64 matmul output lands
  wrong on HW); finisher (dn->rec->bc->mul) emitted a few jc into the
  next window. gpsimd broadcast warmed up at kernel start; gpsimd
  cannot read PSUM and its Q7 adds are ~2x DVE, so it only runs the
  partition broadcasts.
- outputs bf16: real + in-window imag on the SP queue, tail imag on
  the ACT queue.
"""

import numpy as np
import ml_dtypes

import concourse.bacc as bacc
import concourse.tile as tile
from concourse import mybir
from concourse.bass import ts
from concourse.bass_utils import run_bass_kernel_spmd

B, T, D, H = 2, 2048, 1024, 16
HD = 64
NCORE = 8
TP = 4               # head-parallel degree (per batch)
HC = H // TP         # heads per core = 4
C = HC * HD          # local channels = 256
DC = D // 128        # contraction chunks = 8
TQ = T // 128        # 128-row t-chunks = 16
TW = T // 1024       # 1024-col t-chunks = 2
QT = T // 512        # 512-col quarters = 4
XR_SLOTS = 3         # rolling depth of the x quarter buffers
XS_SLOTS = 2

F32 = mybir.dt.float32
BF16 = mybir.dt.bfloat16
EXP = mybir.ActivationFunctionType.Exp

LAST_RESULTS = None
_COMPILED = None


def _build():
    nc = bacc.Bacc("TRN2", target_bir_lowering=False, debug=False,
                   num_devices=NCORE)

    def din(name, shape, dt=BF16):
        return nc.dram_tensor(name, shape, dt, kind="ExternalInput").ap()

    xr_d = din("xrT", [128, DC, T])
    xi_d = din("xiT", [128, DC, T])
    wq = {k: din(f"wq_{k}", [128, DC, C]) for k in ("r", "i", "s")}
    wk = {k: din(f"wk_{k}", [128, DC, C]) for k in ("r", "i", "s")}
    wv = {k: din(f"wv_{k}", [128, DC, C]) for k in ("r", "i", "s")}
    ow = {k: din(f"ow_{k}", [128, 2, D]) for k in ("r", "i", "n")}
    cos_d = din("cos2", [128, T], BF16)
    sin_d = din("sin2", [128, T], BF16)
    outr_d = nc.dram_tensor("out_r", [T, D], BF16, kind="ExternalOutput").ap()
    outi_d = nc.dram_tensor("out_i", [T, D], BF16, kind="ExternalOutput").ap()

    with tile.TileContext(nc) as tc:
        with tc.tile_pool(name="persist", bufs=1) as persist:
            qkcat = persist.tile([128, 2 * HC, T], BF16, name="qkcat")
            vcat = persist.tile([128, TQ, HC, 128], BF16, name="vcat")
            urt = persist.tile([128, 2, T], BF16, name="urt")
            uit = persist.tile([128, 2, T], BF16, name="uit")
            ones = persist.tile([128, 1], BF16, name="ones")
            nc.vector.memset(ones[:], 1.0)
            # dummy broadcast: preloads the gpsimd program while the
            # engine is idle (first dispatch otherwise costs ~7.5us in
            # the middle of the attention phase)
            bwarm_in = persist.tile([1, 8], F32, name="bwarm_in")
            bwarm = persist.tile([128, 8], F32, name="bwarm")
            nc.vector.memset(bwarm_in[:], 1.0)
            nc.gpsimd.partition_broadcast(bwarm[:], bwarm_in[:])

            # -------- input SBUF --------
            xw = tc.alloc_tile_pool(name="xw", bufs=1)
            wqs = {k: xw.tile([128, DC, C], BF16, name=f"wq{k}")
                   for k in ("r", "i", "s")}
            wks = {k: xw.tile([128, DC, C], BF16, name=f"wk{k}")
                   for k in ("r", "i", "s")}
            wvs = {k: xw.tile([128, DC, C], BF16, name=f"wv{k}")
                   for k in ("r", "i", "s")}
            cos = xw.tile([128, T], BF16, name="cos")
            sin = xw.tile([128, T], BF16, name="sin")
            xr = xw.tile([128, XR_SLOTS, DC, 512], BF16, name="xr")
            xi = xw.tile([128, XR_SLOTS, DC, 512], BF16, name="xi")
            xs = xw.tile([128, XS_SLOTS, DC, 512], BF16, name="xs")

            # -------- input DMA: ordered by first consumer --------
            # ACT queue: q weights (first matmul group), then per
            # quarter the xi pieces, with k/v weights slotted after
            # the quarter that gives them enough lead time.
            for dc in range(DC):
                nc.scalar.dma_start(wqs["r"][:, dc:dc + 1],
                                    wq["r"][:, dc:dc + 1])
            nc.scalar.dma_start(wqs["i"][:], wq["i"][:])

            def xi_quarter(q):
                for dc in range(DC):
                    nc.scalar.dma_start(xi[:, q % XR_SLOTS, dc, :],
                                        xi_d[:, dc, ts(q, 512)])

            # NOTE: only the first XR_SLOTS quarters are DMA'd up front.
            # The wrap-around quarter (q=3 reuses slot 0) must be emitted
            # AFTER quarter 0's compute, or Tile wires quarter-0's reads
            # to the later write (last-writer RAW) and the kernel computes
            # on the wrong data.
            xi_quarter(0)
            nc.scalar.dma_start(wqs["s"][:], wq["s"][:])
            for k in ("r", "i", "s"):
                nc.scalar.dma_start(wks[k][:], wk[k][:])
            xi_quarter(1)
            for k in ("r", "i", "s"):
                nc.scalar.dma_start(wvs[k][:], wv[k][:])
            xi_quarter(2)

            # SP queue: xr quarters with their rope-table chunks just
            # behind, o-projection weights at the tail.
            def xr_quarter(q):
                for dc in range(DC):
                    nc.sync.dma_start(xr[:, q % XR_SLOTS, dc, :],
                                      xr_d[:, dc, ts(q, 512)])

            for q in range(XR_SLOTS):
                xr_quarter(q)
                qs = ts(q, 512)
                nc.sync.dma_start(cos[:, qs], cos_d[:, qs])
                nc.sync.dma_start(sin[:, qs], sin_d[:, qs])
            qs = ts(3, 512)
            nc.sync.dma_start(cos[:, qs], cos_d[:, qs])
            nc.sync.dma_start(sin[:, qs], sin_d[:, qs])
            ows = {k: persist.tile([128, 2, D], BF16, name=f"ow{k}")
                   for k in ("r", "i", "n")}
            for k in ("r", "i", "n"):
                nc.sync.dma_start(ows[k][:], ow[k][:])

            # ---------------- projection phase ----------------
            # Karatsuba: m1 = xr@wr, m2 = xi@wi, m3 = xs@ws;
            # re = m1-m2, im = (m3-m1)-m2.
            with tc.tile_pool(name="rt", bufs=1) as rt, \
                 tc.tile_pool(name="pj", bufs=2, space="PSUM") as pj:

                def qk_mrun(pm, wsrc, cc, q, m):
                    slot = q % XR_SLOTS
                    xbuf, wkey = ((xr, "r"), (xi, "i"))[m] if m < 2 \
                        else (None, "s")
                    for dc in range(DC):
                        rhs = (xs[:, q % XS_SLOTS, dc, :] if m == 2
                               else xbuf[:, slot, dc, :])
                        nc.tensor.matmul(pm[:, m, :],
                                         lhsT=wsrc[wkey][:, dc, ts(cc, 128)],
                                         rhs=rhs,
                                         start=(dc == 0),
                                         stop=(dc == DC - 1))

                def qk_group(wsrc, hbase, cc, q, pm=None):
                    hsl = ts(q, 512)
                    if pm is None:
                        pm = pj.tile([128, 3, 512], F32, name="pm")
                        for m in range(3):
                            qk_mrun(pm, wsrc, cc, q, m)
                    # DVE reads at most one PSUM operand per op: stage m2
                    # to SBUF first. NOT on the scalar engine: its
                    # instruction stream sits behind the flow-controlled
                    # DMA issuances and runs ~45us late.
                    qrh = rt.tile([128, 512], BF16, name="qrh")
                    qih = rt.tile([128, 512], BF16, name="qih")
                    uu = rt.tile([128, 512], F32, name="uu")
                    m2c = rt.tile([128, 512], F32, name="m2c")
                    nc.vector.tensor_copy(m2c[:], pm[:, 1, :])
                    nc.vector.tensor_sub(qrh[:], pm[:, 0, :], m2c[:])
                    nc.vector.tensor_sub(uu[:], pm[:, 2, :], m2c[:])
                    nc.vector.tensor_sub(qih[:], uu[:], pm[:, 0, :])
                    # rope
                    h0, h1 = hbase + 2 * cc, hbase + 2 * cc + 1
                    t1 = rt.tile([128, 512], BF16, name="t1")
                    t2 = rt.tile([128, 512], BF16, name="t2")
                    t3 = rt.tile([128, 512], BF16, name="t3")
                    t4 = rt.tile([128, 512], BF16, name="t4")
                    nc.vector.tensor_mul(t1[:], qrh[:], cos[:, hsl])
                    nc.vector.tensor_mul(t2[:], qih[:], sin[:, hsl])
                    nc.vector.tensor_mul(t3[:], qrh[:], sin[:, hsl])
                    nc.vector.tensor_mul(t4[:], qih[:], cos[:, hsl])
                    nc.vector.tensor_sub(qkcat[0:64, h0, hsl],
                                         t1[0:64, :], t2[0:64, :])
                    nc.vector.tensor_sub(qkcat[0:64, h1, hsl],
                                         t1[64:128, :], t2[64:128, :])
                    nc.vector.tensor_add(qkcat[64:128, h0, hsl],
                                         t3[0:64, :], t4[0:64, :])
                    nc.vector.tensor_add(qkcat[64:128, h1, hsl],
                                         t3[64:128, :], t4[64:128, :])

                def v_group(tq):
                    q, k = tq // 4, tq % 4
                    slot = q % XR_SLOTS
                    ksl = ts(k, 128)
                    pm = pj.tile([128, 3, 512], F32, name="pm")
                    vsl = ts(0, 256)
                    for m, xbuf, wkey in ((0, xr, "r"), (1, xi, "i")):
                        for dc in range(DC):
                            nc.tensor.matmul(pm[:, m, vsl],
                                             lhsT=xbuf[:, slot, dc, ksl],
                                             rhs=wvs[wkey][:, dc, :],
                                             start=(dc == 0),
                                             stop=(dc == DC - 1))
                    for dc in range(DC):
                        nc.tensor.matmul(pm[:, 2, vsl],
                                         lhsT=xs[:, q % XS_SLOTS, dc, ksl],
                                         rhs=wvs["s"][:, dc, :],
                                         start=(dc == 0),
                                         stop=(dc == DC - 1))
                    vt = rt.tile([128, C], F32, name="vt")
                    vm2c = rt.tile([128, C], F32, name="vm2c")
                    nc.vector.tensor_copy(vm2c[:], pm[:, 1, vsl])
                    nc.vector.tensor_sub(
                        vcat[:, tq, :, 0:64],
                        pm[:, 0, vsl].rearrange("p (h d) -> p h d", h=HC),
                        vm2c[:].rearrange("p (h d) -> p h d", h=HC))
                    nc.vector.tensor_sub(vt[:], pm[:, 2, vsl], vm2c[:])
                    nc.vector.tensor_sub(
                        vcat[:, tq, :, 64:128],
                        vt[:].rearrange("p (h d) -> p h d", h=HC),
                        pm[:, 0, vsl].rearrange("p (h d) -> p h d", h=HC))

                def xs_adds(q):
                    # all on DVE: gpsimd's slower Q7 adds end up gating
                    # the m3 runs even with early emission
                    for dc in range(DC):
                        nc.vector.tensor_add(xs[:, q % XS_SLOTS, dc, :],
                                             xr[:, q % XR_SLOTS, dc, :],
                                             xi[:, q % XR_SLOTS, dc, :])

                xs_adds(0)
                for q in range(QT):
                    if q == 0:
                        # startup: the first two groups' m1/m2/m3 runs
                        # interleave so the PE has resident-xr work
                        # while xi lands and the xs adds catch up
                        pms = [pj.tile([128, 3, 512], F32, name="pm")
                               for _ in range(2)]
                        for m in range(3):
                            for g in range(2):
                                qk_mrun(pms[g], wqs, g, 0, m)
                        for g in range(2):
                            qk_group(wqs, 0, g, 0, pm=pms[g])
                        xs_adds(1)
                        for cc in range(2):
                            qk_group(wks, HC, cc, 0)
                    else:
                        done = 0
                        for wsrc, hbase in ((wqs, 0), (wks, HC)):
                            for cc in range(2):
                                qk_group(wsrc, hbase, cc, q)
                                done += 1
                                if done == 2 and q + 1 < QT:
                                    # next quarter's xs adds emitted
                                    # mid-quarter so they clear the DVE
                                    # queue before its m3 groups run
                                    xs_adds(q + 1)
                    for k in range(4):
                        v_group(4 * q + k)
                    # rolling-buffer refill: quarter q+XR_SLOTS reuses
                    # this quarter's slot, so its DMA is emitted only now
                    if q + XR_SLOTS < QT:
                        xr_quarter(q + XR_SLOTS)
                        xi_quarter(q + XR_SLOTS)

            # x and q/k/v weights are consumed; free their SBUF before
            # opening the attention pools.
            xw.release()

            # PSUM pools stack-ordered so mm/avp (released before the
            # o-proj tail) sit on top
            op = tc.alloc_tile_pool(name="op", bufs=2, space="PSUM")
            avp = tc.alloc_tile_pool(name="avp", bufs=1, space="PSUM")
            mm = tc.alloc_tile_pool(name="mm", bufs=2, space="PSUM")
            att = tc.alloc_tile_pool(name="att", bufs=6)
            asm = tc.alloc_tile_pool(name="asm", bufs=2)
            ost = tc.alloc_tile_pool(name="ost", bufs=4)

            # ---------------- attention + o-projection ----------------
            # iw-outer / h-inner: after window iw=0 finishes all 4 heads,
            # the o-projection for query rows 0:1024 is unblocked; its
            # 64 single-bank PSUM pieces are interleaved into iw=1's jc
            # stream to fill the PE slack under the exp-bound window.
            # The per-window softmax finisher (dn -> rec -> bc -> muls)
            # is emitted a few jc iterations INTO the next window so the
            # chain pipelines under exp.
            pend = None  # finisher state of the previous window

            def fin_dn(p):
                # one [128,512] op-ring slot per 512-half, each written
                # at partition 0 (a partition-64 matmul output lands
                # wrong on HW). Same name/shape as the o-proj pieces so
                # the pool keeps a single 2-buffer ring.
                dns = []
                for half in range(2):
                    dn = op.tile([128, 512], F32, name="po")
                    nc.tensor.matmul(dn[0:1, :], lhsT=ones[:],
                                     rhs=p["esum"][:, ts(half, 512)],
                                     start=True, stop=True)
                    dns.append(dn)
                p["dn"] = dns

            def fin_rec(p):
                rec = asm.tile([1, 1024], F32, name="rec")
                for half in range(2):
                    nc.vector.reciprocal_approx_fast(
                        rec[:, ts(half, 512)], p["dn"][half][0:1, :])
                p["rec"] = rec

            def fin_bc(p):
                bc = asm.tile([128, 1024], F32, name="bc")
                nc.gpsimd.partition_broadcast(bc[:], p["rec"][:])
                p["bc"] = bc

            def fin_mul(p):
                ucc, up0, isl = p["ucc"], p["up0"], p["isl"]
                nc.vector.tensor_mul(urt[up0:up0 + 64, ucc, isl],
                                     p["avr"][0:64, :], p["bc"][0:64, :])
                nc.vector.tensor_mul(uit[up0:up0 + 64, ucc, isl],
                                     p["avr"][64:128, :], p["bc"][64:128, :])

            opool = [None]

            def o_piece(tq, oc, im):
                def emit(tail=False):
                    tslq, osl = ts(tq, 128), ts(oc, 512)
                    po = opool[0].tile([128, 512], F32, name="po")
                    if im == 0:
                        seq = ((urt, 0, "r"), (urt, 1, "r"),
                               (uit, 0, "n"), (uit, 1, "n"))
                        dst = outr_d
                    else:
                        seq = ((urt, 0, "i"), (urt, 1, "i"),
                               (uit, 0, "r"), (uit, 1, "r"))
                        dst = outi_d
                    for n, (ut, ch, wk2) in enumerate(seq):
                        nc.tensor.matmul(po[:], lhsT=ut[:, ch, tslq],
                                         rhs=ows[wk2][:, ch, osl],
                                         start=(n == 0), stop=(n == 3))
                    stp = ost.tile([128, 512], BF16, name="stp")
                    nc.vector.tensor_copy(stp[:], po[:])
                    # imag tail pieces drain on the (then idle) ACT
                    # queue; everything else on SP
                    if im == 1 and tq >= 8:
                        nc.scalar.dma_start(dst[tslq, osl], stp[:])
                    else:
                        nc.sync.dma_start(dst[tslq, osl], stp[:])
                return emit

            OSLOT = {4, 6, 8, 10, 12, 14}
            opq = []
            opool[0] = op
            for iw in range(TW):
                isl = ts(iw, 1024)
                if iw == 1:
                    opq = [o_piece(tq, oc, im)
                           for tq in range(8) for im in range(2)
                           for oc in range(2)]
                for h in range(HC):
                    ucc, up0 = h // 2, (h % 2) * 64
                    av = avp.tile([128, 1024], F32, name="av")
                    esum = asm.tile([128, 1024], BF16, name="esum")
                    for jc in range(TQ):
                        s = mm.tile([128, 1024], F32, name="mmt")
                        for half in range(2):
                            nc.tensor.matmul(
                                s[:, ts(half, 512)],
                                lhsT=qkcat[:, HC + h, ts(jc, 128)],
                                rhs=qkcat[:, h, ts(2 * iw + half, 512)],
                                start=True, stop=True)
                        es = att.tile([128, 1024], BF16, name="es")
                        nc.scalar.activation(es[:], s[:], EXP, scale=0.125)
                        for half in range(2):
                            psl = ts(half, 512)
                            nc.tensor.matmul(av[:, psl],
                                             lhsT=vcat[:, jc, h, :],
                                             rhs=es[:, psl],
                                             start=(jc == 0),
                                             stop=(jc == TQ - 1))
                        if jc == 0:
                            nc.vector.tensor_copy(esum[:], es[:])
                        else:
                            nc.vector.tensor_add(esum[:], esum[:], es[:])
                        if pend is not None:
                            if jc == 1:
                                fin_dn(pend)
                            elif jc == 2:
                                fin_rec(pend)
                            elif jc == 3:
                                fin_bc(pend)
                            elif jc == 5:
                                fin_mul(pend)
                                pend = None
                        if opq and jc in OSLOT and (h > 0 or jc >= 8):
                            opq.pop(0)()
                    avr = asm.tile([128, 1024], BF16, name="avr")
                    nc.vector.tensor_copy(avr[:], av[:])
                    pend = {"esum": esum, "avr": avr,
                            "ucc": ucc, "up0": up0, "isl": isl}
            # flush the final window's finisher, then the second-half
            # o-projection (query rows 1024:2048)
            fin_dn(pend)
            fin_rec(pend)
            fin_bc(pend)
            fin_mul(pend)
            pend = None
            # tail o-projection: scores/av PSUM is dead, so re-use its
            # banks as a deeper 4-slot ring (drain-WAR no longer paces
            # the PE)
            mm.release()
            avp.release()
            otail = tc.alloc_tile_pool(name="otail", bufs=4, space="PSUM")
            opool[0] = otail
            while opq:          # pieces that found no insertion slot
                opq.pop(0)(True)
            for tq in range(8, 16):
                for im in range(2):
                    for oc in range(2):
                        o_piece(tq, oc, im)(True)

            ost.release()
            asm.release()
            att.release()
            otail.release()
            op.release()

    nc.compile()
    return nc


def _to_bf16_kxm(arr, parts=128):
    """[K, M] fp32 -> [128, K//128, M] bf16 with K split as (chunk, part)."""
    k, m = arr.shape
    out = arr.reshape(k // parts, parts, m).transpose(1, 0, 2)
    return np.ascontiguousarray(out.astype(ml_dtypes.bfloat16))


def _rope_tables():
    inv_freq = 1.0 / (10000.0 ** (np.arange(0, HD, 2, dtype=np.float64) / HD))
    invf64 = np.concatenate([inv_freq, inv_freq])          # [64]
    ang = invf64[:, None] * np.arange(T, dtype=np.float64)[None, :]  # [64, T]
    cos2 = np.tile(np.cos(ang), (2, 1)).astype(ml_dtypes.bfloat16)
    sin2 = np.tile(np.sin(ang), (2, 1)).astype(ml_dtypes.bfloat16)
    return np.ascontiguousarray(cos2), np.ascontiguousarray(sin2)


def kernel(x_real, x_imag, q_wr, q_wi, k_wr, k_wi, v_wr, v_wi, o_wr, o_wi):
    global _COMPILED, LAST_RESULTS
    if _COMPILED is None:
        _COMPILED = _build()
    nc = _COMPILED

    cos2, sin2 = _rope_tables()
    xt = {}
    for b in range(B):
        xt[("r", b)] = _to_bf16_kxm(np.asarray(x_real[b]).T.astype(np.float32))
        xt[("i", b)] = _to_bf16_kxm(np.asarray(x_imag[b]).T.astype(np.float32))

    in_maps = []
    for core in range(NCORE):
        b, g = core // TP, core % TP
        cols = slice(g * C, (g + 1) * C)
        m = {"xrT": xt[("r", b)], "xiT": xt[("i", b)],
             "cos2": cos2, "sin2": sin2}
        for nm, wr_, wi_ in (("wq", q_wr, q_wi), ("wk", k_wr, k_wi),
                             ("wv", v_wr, v_wi)):
            wrc = np.asarray(wr_[:, cols])
            wic = np.asarray(wi_[:, cols])
            m[f"{nm}_r"] = _to_bf16_kxm(wrc)
            m[f"{nm}_i"] = _to_bf16_kxm(wic)
            m[f"{nm}_s"] = _to_bf16_kxm(wrc + wic)
        m["ow_r"] = _to_bf16_kxm(np.asarray(o_wr[cols, :]))
        m["ow_i"] = _to_bf16_kxm(np.asarray(o_wi[cols, :]))
        m["ow_n"] = _to_bf16_kxm(-np.asarray(o_wi[cols, :]))
        in_maps.append(m)

    res = run_bass_kernel_spmd(nc, in_maps, core_ids=list(range(NCORE)))
    LAST_RESULTS = res

    final_r = np.zeros((B, T, D), np.float32)
    final_i = np.zeros((B, T, D), np.float32)
    for core in range(NCORE):
        b = core // TP
        final_r[b] += np.asarray(res.results[core]["out_r"],
                                 dtype=np.float32)
        final_i[b] += np.asarray(res.results[core]["out_i"],
                                 dtype=np.float32)
    return final_r, final_i
